# revision 1
# baseline (speedup 1.0000x reference)
"""Trainium2 Bass kernel for nn_EvenLayer (LDPC min-sum check-node update).

Reference semantics (B=8 batches, E=3600 edges):
    neighbor = inf_mask == 0            # (E, E)
    signs    = sign(prod(where(neighbor, x, 1), axis=-1))
    mins     = min(|x| + inf_mask, axis=-1)
    out      = signs * max(mins - bias, 0)

The mask encodes "shares a check node, excluding self" — an equivalence
relation minus the diagonal. The host verifies that structure at runtime
(values only {0, +inf}, empty diagonal, rows = leader-equality classes);
on success each edge-group (check node, size d=6) is packed into slots,
sharded over the 8 cores, and a small SPMD kernel computes per slot:
    loo_min  = leave-one-out min of |x| over the group  (tournament tree)
    loo_sgn  = sign(leave-one-out product)              (tournament tree)
    out      = relu(loo_min - bias) * loo_sgn
which is bit-exact vs the reference (including sign(0) = 0 for inputs
containing exact zeros).

The fast path (d=6) splits compute across two engines running in
parallel after one input DMA: the DVE runs the leave-one-out min
tournament (the first level as a single tensor_reduce(min,
apply_absolute_value=True)), M - bias, and a fused scalar_tensor_tensor
tail computing relu(M - b) * sgn in one op; the GPSIMD engine
concurrently computes |x| = x*sign(x) for the tournament's last level,
the leave-one-out product tournament, and sgn = (p > 0) - (p < 0)
(GPSIMD only supports fp add/mult/sub/compare on real TRN2 — bitwise
and min/max are DVE-only, and |x| via compares+mult keeps the abs off
the DVE critical path).
The module is built as raw Bacc engine streams (no Block), which drops
the block entry/exit barrier rounds.

If the mask is not group-structured, a generic dense kernel computes the
masked reductions directly from the mask data (including the reference's
product-underflow semantics for signs). A legacy DMA-based group kernel
covers group sizes d != 6.
"""

import contextlib

import numpy as np

B, E, NCORES = 8, 3600, 8

_NC_CACHE = {}
TRACE = False
LAST_RESULT = None  # BassKernelResults of the last run (for test harness)


def _analyze(inf_mask):
    """Return leader labels if the mask is exactly an equivalence relation
    minus the diagonal with values {0, +inf}; else None."""
    m = np.asarray(inf_mask)
    if m.ndim != 2 or m.shape[0] != m.shape[1]:
        return None
    if not np.all((m == 0) | np.isposinf(m)):
        return None
    nb = m == 0
    if nb.diagonal().any():
        return None
    n = m.shape[0]
    idx = np.arange(n)
    first = np.argmax(nb, axis=1)
    has = nb.any(axis=1)
    leader = np.where(has, np.minimum(idx, first), idx)
    eq = leader[:, None] == leader[None, :]
    np.fill_diagonal(eq, False)
    if not np.array_equal(nb, eq):
        return None
    return leader


def _build_slots(leader, nbatch=B):
    """Pack groups into (NCORES, blocks, gpb, d) slot->edge index array (-1 pad)."""
    max_blocks = max(128 // nbatch, 1)
    order = np.argsort(leader, kind="stable")
    lead_sorted = leader[order]
    uniq, counts = np.unique(lead_sorted, return_counts=True)
    G = len(uniq)
    d = max(int(counts.max()), 2)
    G8 = ((G + NCORES - 1) // NCORES) * NCORES
    slot_edge = np.full((G8, d), -1, dtype=np.int64)
    col = np.concatenate([np.arange(c) for c in counts])
    row = np.repeat(np.arange(G), counts)
    slot_edge[row, col] = order
    Gc = G8 // NCORES
    gpb = (Gc + max_blocks - 1) // max_blocks   # groups per partition-block
    blocks = (Gc + gpb - 1) // gpb
    Gcp = blocks * gpb
    slot_all = slot_edge.reshape(NCORES, Gc, d)
    if Gcp != Gc:
        pad = np.full((NCORES, Gcp - Gc, d), -1, dtype=np.int64)
        slot_all = np.concatenate([slot_all, pad], axis=1)
    return slot_all.reshape(NCORES, blocks, gpb, d), d, blocks, gpb


def _pack_xb(x, bias, slot_core, P, F, EP):
    """Pack one core's slots into its padded (P, EP) input rows."""
    Bn = x.shape[0]
    e = slot_core                        # (blocks, gpb, d)
    valid = e >= 0
    ec = np.clip(e, 0, None)
    xs = np.where(valid[None], x[:, ec], np.float32(np.inf))
    bsv = np.where(valid, bias[0, ec], np.float32(0.0))
    bsv = np.broadcast_to(bsv[None], (Bn,) + bsv.shape)
    xb = np.zeros((P, EP), np.float32)
    xb[:, :F] = xs.reshape(P, F)
    xb[:, F : 2 * F] = bsv.reshape(P, F)
    return xb


def _build_fast_nc(P, F, gpb, d):
    """Optimized group kernel (d == 6): HWDGE DMA in/out; min tree + fused
    relu*sign tail on DVE, product tree + sign construction on GPSIMD.

    Built as a Bacc module (raw engine streams, no Block) so walrus lowers
    it; every op is semaphore-chained (the race detector requires semaphore
    ordering even within one engine). The GPSIMD engine only gets fp
    add/mult/subtract/compare ops (bitwise and min/max are DVE-only on
    real TRN2).
    """
    import concourse.bass as bass
    from concourse import bacc, mybir
    from concourse._compat import get_trn_type

    assert d == 6
    f32 = mybir.dt.float32
    i32 = mybir.dt.int32
    AL = mybir.AluOpType
    AX = mybir.AxisListType

    nc = bacc.Bacc(get_trn_type() or "TRN2")
    xb = nc.declare_dram_parameter("xb", [P, 2 * F], f32, isOutput=False)
    ys = nc.declare_dram_parameter("ys", [P, F], f32, isOutput=True)

    with contextlib.ExitStack() as ctx:
        XB = ctx.enter_context(nc.sbuf_tensor("XB", [P, 2 * F], f32))
        A = ctx.enter_context(nc.sbuf_tensor("A", [P, F], f32))
        W = ctx.enter_context(nc.sbuf_tensor("W", [P, gpb, 8], f32))
        T = ctx.enter_context(nc.sbuf_tensor("T", [P, gpb, 8], f32))
        TP = ctx.enter_context(nc.sbuf_tensor("TP", [P, F], f32))
        M = ctx.enter_context(nc.sbuf_tensor("M", [P, F], f32))
        R = ctx.enter_context(nc.sbuf_tensor("R", [P, F], f32))
        SGN = ctx.enter_context(nc.sbuf_tensor("SGN", [P, F], f32))
        SG2 = ctx.enter_context(nc.sbuf_tensor("SG2", [P, F], f32))
        O = ctx.enter_context(nc.sbuf_tensor("O", [P, F], f32))
        s_in = ctx.enter_context(nc.semaphore("s_in"))
        s_out = ctx.enter_context(nc.semaphore("s_out"))
        s_g = ctx.enter_context(nc.semaphore("s_g"))
        s_v = ctx.enter_context(nc.semaphore("s_v"))

        X = XB[:, 0:F]
        Bt = XB[:, F : 2 * F]
        X3 = X.rearrange("p (g d) -> p g d", d=d)
        X4 = X.rearrange("p (g k two) -> p g k two", two=2, k=3)
        M3 = M[:].rearrange("p (g d) -> p g d", d=d)
        A3 = A[:].rearrange("p (g d) -> p g d", d=d)

        def wsv(WS, off, apdims):
            base = WS[:, :, :]
            return bass.AP(WS, base.offset + off,
                           [base.ap[0], base.ap[1]] + apdims)

        def xsv(off, apdims):
            return bass.AP(XB, X3.offset + off,
                           [X3.ap[0], X3.ap[1]] + apdims)

        def asv(off, apdims):
            return bass.AP(A, A3.offset + off,
                           [A3.ap[0], A3.ap[1]] + apdims)

        # SP: input DMA, then output DMA once the final op lands
        nc.sync.dma_start(out=XB[:], in_=xb[:]).then_inc(s_in, 16)

        # GPSIMD: leave-one-out product tournament and sgn = +-1.0
        g = nc.gpsimd
        n = [0]

        def ginc(inst):
            inst.then_inc(s_g, 1)
            n[0] += 1
            return n[0]

        g.wait_ge(s_in, 16)
        # A = |x| = x * sign(x) (Pool-legal: compares/sub/mult only); the
        # possible -0.0 for x = -0.0 is value-equal through the min tree and
        # forced back to +0 by the final relu
        t_xgt = ginc(g.tensor_single_scalar(
            out=A[:], in_=X, scalar=0.0, op=AL.is_gt))
        t_xlt = ginc(g.tensor_single_scalar(
            out=SG2[:], in_=X, scalar=0.0, op=AL.is_lt))
        g.wait_ge(s_g, t_xlt)
        t_sgx = ginc(g.tensor_tensor(
            out=A[:], in0=A[:], in1=SG2[:], op=AL.subtract))
        g.wait_ge(s_g, t_sgx)
        t_abs = ginc(g.tensor_tensor(
            out=A[:], in0=A[:], in1=X, op=AL.mult))
        t_p1 = ginc(g.tensor_tensor(
            out=wsv(T, 0, [[1, 3]]), in0=xsv(0, [[2, 3]]),
            in1=xsv(1, [[2, 3]]), op=AL.mult))
        g.wait_ge(s_g, t_p1)
        t_p2a = ginc(g.tensor_tensor(
            out=wsv(T, 3, [[1, 2]]), in0=wsv(T, 1, [[-1, 2]]),
            in1=wsv(T, 2, [[0, 2]]), op=AL.mult))
        t_p2b = ginc(g.tensor_tensor(
            out=wsv(T, 5, [[1, 1]]), in0=wsv(T, 0, [[1, 1]]),
            in1=wsv(T, 1, [[1, 1]]), op=AL.mult))
        g.wait_ge(s_g, t_p2b)
        t_p3 = ginc(g.tensor_tensor(
            out=TP[:].rearrange("p (g d) -> p g d", d=d),
            in0=xsv(1, [[2, 3], [-1, 2]]),
            in1=wsv(T, 3, [[1, 3], [0, 2]]), op=AL.mult))
        g.wait_ge(s_g, t_p3)
        # sgn = (TP > 0) - (TP < 0): exactly sign(), including sign(0) = 0
        t_gt = ginc(g.tensor_single_scalar(
            out=SGN[:], in_=TP[:], scalar=0.0, op=AL.is_gt))
        t_lt = ginc(g.tensor_single_scalar(
            out=SG2[:], in_=TP[:], scalar=0.0, op=AL.is_lt))
        g.wait_ge(s_g, t_lt)
        t_sgn = ginc(g.tensor_tensor(
            out=SGN[:], in0=SGN[:], in1=SG2[:], op=AL.subtract))

        # DVE: |x|, leave-one-out min tournament, out = relu(M - b) * sgn
        v = nc.vector
        m = [0]

        def vinc(inst):
            inst.then_inc(s_v, 1)
            m[0] += 1
            return m[0]

        nc.vector.wait_ge(s_in, 16)
        t_l1 = vinc(v.tensor_reduce(
            out=W[:, :, 0:3], in_=X4, axis=AX.X, op=AL.min,
            apply_absolute_value=True))
        nc.vector.wait_ge(s_v, m[0])
        t_l2a = vinc(v.tensor_tensor(
            out=wsv(W, 3, [[1, 2]]), in0=wsv(W, 1, [[-1, 2]]),
            in1=wsv(W, 2, [[0, 2]]), op=AL.min))
        t_l2b = vinc(v.tensor_tensor(
            out=wsv(W, 5, [[1, 1]]), in0=wsv(W, 0, [[1, 1]]),
            in1=wsv(W, 1, [[1, 1]]), op=AL.min))
        nc.vector.wait_ge(s_v, m[0])
        nc.vector.wait_ge(s_g, t_abs)
        t_l3 = vinc(v.tensor_tensor(
            out=M3, in0=asv(1, [[2, 3], [-1, 2]]),
            in1=wsv(W, 3, [[1, 3], [0, 2]]), op=AL.min))
        nc.vector.wait_ge(s_v, m[0])
        t_sub = vinc(v.tensor_sub(R[:], M[:], Bt))
        nc.vector.wait_ge(s_v, m[0])
        nc.vector.wait_ge(s_g, t_sgn)
        t_fin = vinc(v.scalar_tensor_tensor(
            out=O[:], in0=R[:], scalar=0.0, in1=SGN[:],
            op0=AL.max, op1=AL.mult))

        nc.sync.wait_ge(s_v, t_fin)
        nc.sync.dma_start(out=ys[:], in_=O[:]).then_inc(s_out, 16)
        nc.sync.wait_ge(s_out, 16)
    nc.finalize()
    return nc


def _run_spmd(nc, in_maps):
    global LAST_RESULT
    from concourse.bass_utils import run_bass_kernel_spmd

    res = run_bass_kernel_spmd(
        nc, in_maps, core_ids=list(range(NCORES)), trace=TRACE
    )
    LAST_RESULT = res
    return res.results


def _kernel_fast(x, bias, leader):
    Bn, E_ = x.shape
    slot_all, d, blocks, gpb = _build_slots(leader, nbatch=Bn)
    P, F = Bn * blocks, gpb * d
    if d != 6 or P > 128:
        return _kernel_fast_legacy(x, bias, slot_all, d, blocks, gpb)
    key = ("fast6", P, F, gpb, d)
    if key not in _NC_CACHE:
        _NC_CACHE[key] = _build_fast_nc(P, F, gpb, d)
    nc = _NC_CACHE[key]

    in_maps = []
    for c in range(NCORES):
        e = slot_all[c]
        valid = e >= 0
        ec = np.clip(e, 0, None)
        xs = np.where(valid[None], x[:, ec], np.float32(np.inf))
        bsv = np.where(valid, bias[0, ec], np.float32(0.0))
        bsv = np.broadcast_to(bsv[None], (Bn,) + bsv.shape)
        xbv = np.concatenate([xs.reshape(P, F), bsv.reshape(P, F)], axis=1)
        in_maps.append({"xb": np.ascontiguousarray(xbv, np.float32)})

    results = _run_spmd(nc, in_maps)

    out = np.empty((Bn, E_), np.float32)
    for c in range(NCORES):
        e = slot_all[c]
        valid = e >= 0
        ysv = results[c]["ys"].reshape(Bn, blocks, gpb, d)
        out[:, e[valid]] = ysv[:, valid]
    return out


def kernel(inputs, bias, inf_mask):
    x = np.ascontiguousarray(np.asarray(inputs), np.float32)
    bias = np.ascontiguousarray(np.asarray(bias), np.float32)
    inf_mask = np.asarray(inf_mask)

    leader = _analyze(inf_mask)
    if leader is not None:
        return _kernel_fast(x, bias, leader)
    return _kernel_dense(x, bias, inf_mask)


# ---------------------------------------------------------------------------
# Legacy group kernel (any d): single DMA in / compute on DVE / DMA out.
# ---------------------------------------------------------------------------

def _build_legacy_nc(P, F, gpb, d):
    import concourse.bass as bass
    from concourse import mybir

    f32 = mybir.dt.float32
    i32 = mybir.dt.int32
    AL = mybir.AluOpType

    nc = bass.Bass()
    xb = nc.declare_dram_parameter("xb", [P, 2 * F], f32, isOutput=False)
    ys = nc.declare_dram_parameter("ys", [P, F], f32, isOutput=True)

    with contextlib.ExitStack() as ctx:
        XB = ctx.enter_context(nc.sbuf_tensor("XB", [P, 2 * F], f32))
        A = ctx.enter_context(nc.sbuf_tensor("A", [P, F], f32))
        T = ctx.enter_context(nc.sbuf_tensor("T", [P, F], f32))
        Ti = ctx.enter_context(nc.sbuf_tensor("Ti", [P, F], i32))
        Km = ctx.enter_context(nc.sbuf_tensor("Km", [P, F], i32))
        Kp = ctx.enter_context(nc.sbuf_tensor("Kp", [P, F], i32))
        M = ctx.enter_context(nc.sbuf_tensor("M", [P, F], f32))
        Wb = ctx.enter_context(nc.sbuf_tensor("Wb", [P, gpb, max(d - 2, 1), 2], f32))
        Wp = ctx.enter_context(nc.sbuf_tensor("Wp", [P, gpb, max(d - 2, 1), 2], f32))
        R = ctx.enter_context(nc.sbuf_tensor("R", [P, F], f32))
        O = ctx.enter_context(nc.sbuf_tensor("O", [P, F], i32))

        s_in = ctx.enter_context(nc.semaphore("s_in"))
        s_dve = ctx.enter_context(nc.semaphore("s_dve"))
        s_out = ctx.enter_context(nc.semaphore("s_out"))
        s_v = ctx.enter_context(nc.semaphore("s_v"))
        block = ctx.enter_context(nc.Block())

        X = XB[:, 0:F]
        Bt = XB[:, F : 2 * F]

        @block.sync
        def _(sync):
            sync.dma_start(out=XB[:], in_=xb[:]).then_inc(s_in, 16)
            sync.wait_ge(s_dve, 1)
            sync.dma_start(out=ys[:], in_=O[:].bitcast(f32)).then_inc(s_out, 16)
            sync.wait_ge(s_out, 16)

        @block.vector
        def _(vector):
            X3 = X.rearrange("p (g d) -> p g d", d=d)
            A3 = A[:].rearrange("p (g d) -> p g d", d=d)
            M3 = M[:].rearrange("p (g d) -> p g d", d=d)
            T3 = T[:].rearrange("p (g d) -> p g d", d=d)

            cnt = [0]
            waited = [0]

            def emit(fn, wait=None):
                if wait is None:
                    wait = cnt[0]
                if wait > waited[0]:
                    vector.wait_ge(s_v, wait)
                    waited[0] = wait
                fn().then_inc(s_v, 1)
                cnt[0] += 1
                return cnt[0]

            def tt(out, a, b, op, wait=None):
                return emit(
                    lambda: nc.vector.tensor_tensor(out=out, in0=a, in1=b, op=op),
                    wait=wait,
                )

            def loo_chain(src_h, src3, out_h, out3, wb_h, op, first_wait):
                soff = src3.offset
                pstep, gstep = src3.ap[0], src3.ap[1]
                ooff = out3.offset
                opp, opg = out3.ap[0], out3.ap[1]

                def sv(off, apdims):
                    return bass.AP(src_h, soff + off, [pstep, gstep] + apdims)

                if d == 2:
                    emit(
                        lambda: nc.vector.tensor_copy(out3, sv(1, [[-1, 2]])),
                        wait=first_wait,
                    )
                    return
                if d == 4:
                    t0 = tt(wb_h[:, :, 0, :], sv(0, [[2, 2]]), sv(1, [[2, 2]]), op,
                            wait=first_wait)
                    wb4 = wb_h[:, :, :, :]
                    mp_swap_b = bass.AP(wb_h, wb4.offset + 1,
                                        [wb4.ap[0], wb4.ap[1], [-1, 2], [0, 2]])
                    tt(bass.AP(out_h, ooff, [opp, opg, [2, 2], [1, 2]]),
                       sv(1, [[2, 2], [-1, 2]]), mp_swap_b, op, wait=t0)
                    return

                def U(k):
                    return sv(k, [[d - 1 - 2 * k, 2]])

                wb4 = wb_h[:, :, :, :]
                prev_t = emit(
                    lambda: nc.vector.tensor_copy(wb_h[:, :, 0, :], U(0)),
                    wait=first_wait,
                )
                for k in range(1, d - 2):
                    prev_t = tt(wb_h[:, :, k, :], wb_h[:, :, k - 1, :], U(k), op,
                                wait=prev_t)
                ends = bass.AP(out_h, ooff + d - 1, [opp, opg, [-(d - 1), 2]])
                tt(ends, wb_h[:, :, d - 3, :], U(d - 2), op, wait=prev_t)
                pre_view = bass.AP(wb_h, wb4.offset, [wb4.ap[0], wb4.ap[1], [2, d - 2]])
                suf_rev = bass.AP(wb_h, wb4.offset + (d - 3) * 2 + 1,
                                  [wb4.ap[0], wb4.ap[1], [-2, d - 2]])
                tt(out3[:, :, 1 : d - 1], pre_view, suf_rev, op)

            emit(lambda: nc.vector.memset(Km[:], -2147483648), wait=0)
            t_msets = emit(lambda: nc.vector.memset(Kp[:], 2147483647), wait=0)

            vector.wait_ge(s_in, 16)
            t_abs = emit(
                lambda: nc.vector.tensor_tensor(
                    out=A[:].bitcast(i32), in0=X.bitcast(i32), in1=Kp[:],
                    op=AL.bitwise_and
                ),
                wait=t_msets,
            )
            loo_chain(XB, X3, T, T3, Wp, AL.mult, first_wait=0)
            t_prod = cnt[0]
            loo_chain(A, A3, M, M3, Wb, AL.min, first_wait=t_abs)
            t_min = cnt[0]

            t_sub = emit(lambda: nc.vector.tensor_sub(R[:], M[:], Bt), wait=t_min)
            emit(lambda: nc.vector.tensor_tensor(
                out=Ti[:], in0=T[:].bitcast(i32), in1=Km[:], op=AL.bitwise_and),
                wait=t_prod)
            emit(lambda: nc.vector.tensor_relu(out=R[:], in_=R[:]), wait=t_sub)
            vector.wait_ge(s_v, cnt[0])
            nc.vector.tensor_tensor(
                out=O[:], in0=R[:].bitcast(i32), in1=Ti[:], op=AL.bitwise_or
            ).then_inc(s_dve, 1)

    return nc


def _kernel_fast_legacy(x, bias, slot_all, d, blocks, gpb):
    Bn = x.shape[0]
    P, F = Bn * blocks, gpb * d
    key = ("legacy", P, F, gpb, d)
    if key not in _NC_CACHE:
        _NC_CACHE[key] = _build_legacy_nc(P, F, gpb, d)
    nc = _NC_CACHE[key]

    in_maps = []
    for c in range(NCORES):
        e = slot_all[c]
        valid = e >= 0
        ec = np.clip(e, 0, None)
        xs = np.where(valid[None], x[:, ec], np.float32(np.inf))
        bsv = np.where(valid, bias[0, ec], np.float32(0.0))
        bsv = np.broadcast_to(bsv[None], (Bn,) + bsv.shape)
        xbv = np.concatenate([xs.reshape(P, F), bsv.reshape(P, F)], axis=1)
        in_maps.append({"xb": np.ascontiguousarray(xbv, np.float32)})

    results = _run_spmd(nc, in_maps)

    out = np.empty(x.shape, np.float32)
    for c in range(NCORES):
        e = slot_all[c]
        valid = e >= 0
        ysv = results[c]["ys"].reshape(Bn, blocks, gpb, d)
        out[:, e[valid]] = ysv[:, valid]
    return out


# ---------------------------------------------------------------------------
# Dense fallback (arbitrary masks), unchanged from the baseline.
# ---------------------------------------------------------------------------

def _build_dense_nc(Bn, E_, Ec):
    import concourse.bass as bass
    from concourse import mybir

    f32 = mybir.dt.float32
    AL = mybir.AluOpType
    AX = mybir.AxisListType

    PT = 128
    ntiles = (Ec + PT - 1) // PT
    assert Ec % ntiles == 0 and (Ec // ntiles) <= PT
    TR = Ec // ntiles  # rows per tile

    nc = bass.Bass()
    mrows = nc.declare_dram_parameter("mrows", [Ec, E_], f32, isOutput=False)
    xfull = nc.declare_dram_parameter("xfull", [Bn, E_], f32, isOutput=False)
    brows = nc.declare_dram_parameter("brows", [Ec, 1], f32, isOutput=False)
    ys = nc.declare_dram_parameter("ys", [Ec, Bn], f32, isOutput=True)

    with contextlib.ExitStack() as ctx:
        XB = []
        for b in range(Bn):
            XB.append(ctx.enter_context(nc.sbuf_tensor(f"XBc{b}", [TR, E_], f32)))
        MT = ctx.enter_context(nc.sbuf_tensor("MT", [TR, E_], f32))
        W = ctx.enter_context(nc.sbuf_tensor("W", [TR, E_], f32))
        SC = ctx.enter_context(nc.sbuf_tensor("SC", [TR, E_], f32))
        SC2 = ctx.enter_context(nc.sbuf_tensor("SC2", [TR, E_], f32))
        BC = ctx.enter_context(nc.sbuf_tensor("BC", [TR, 1], f32))
        MI = ctx.enter_context(nc.sbuf_tensor("MI", [TR, 1], f32))
        SG = ctx.enter_context(nc.sbuf_tensor("SG", [TR, 1], f32))
        PR = ctx.enter_context(nc.sbuf_tensor("PR", [TR, 1], f32))
        OT = ctx.enter_context(nc.sbuf_tensor("OT", [TR, Bn], f32))

        s_bc = ctx.enter_context(nc.semaphore("s_bc"))
        s_m = ctx.enter_context(nc.semaphore("s_m"))
        s_v = ctx.enter_context(nc.semaphore("s_v"))
        s_t = ctx.enter_context(nc.semaphore("s_t"))
        s_out = ctx.enter_context(nc.semaphore("s_out"))
        block = ctx.enter_context(nc.Block())

        @block.sync
        def _(sync):
            for b in range(Bn):
                src = bass.AP(xfull, b * E_, [[0, TR], [1, E_]])
                sync.dma_start(out=XB[b][:], in_=src).then_inc(s_bc, 16)
            for t in range(ntiles):
                if t:
                    sync.wait_ge(s_t, t)
                    sync.dma_start(
                        out=ys[(t - 1) * TR : t * TR, :], in_=OT[:]
                    ).then_inc(s_out, 16)
                sync.dma_start(out=MT[:], in_=mrows[t * TR : (t + 1) * TR, :]).then_inc(s_m, 16)
                sync.dma_start(out=BC[:], in_=brows[t * TR : (t + 1) * TR, :]).then_inc(s_m, 16)
            sync.wait_ge(s_t, ntiles)
            sync.dma_start(
                out=ys[(ntiles - 1) * TR : ntiles * TR, :], in_=OT[:]
            ).then_inc(s_out, 16)
            sync.wait_ge(s_out, 16 * ntiles)

        @block.vector
        def _(vector):
            cnt = [0]
            waited = [0]

            def emit(fn, wait=None):
                if wait is None:
                    wait = cnt[0]
                if wait > waited[0]:
                    vector.wait_ge(s_v, wait)
                    waited[0] = wait
                fn().then_inc(s_v, 1)
                cnt[0] += 1
                return cnt[0]

            vector.wait_ge(s_bc, 16 * Bn)
            for t in range(ntiles):
                vector.wait_ge(s_m, 32 * (t + 1))
                if t:
                    vector.wait_ge(s_out, 16 * t)
                emit(lambda: nc.vector.tensor_single_scalar(
                    out=W[:], in_=MT[:], scalar=0.0, op=AL.is_equal))
                for b in range(Bn):
                    emit(lambda b=b: nc.vector.tensor_scalar_mul(SC2[:], XB[b][:], -1.0))
                    emit(lambda b=b: nc.vector.tensor_max(SC2[:], SC2[:], XB[b][:]))
                    emit(lambda: nc.vector.tensor_add(SC[:], MT[:], SC2[:]))
                    emit(lambda: nc.vector.tensor_reduce(
                        out=MI[:], in_=SC[:], axis=AX.X, op=AL.min))
                    emit(lambda b=b: nc.vector.tensor_scalar_add(SC[:], XB[b][:], -1.0))
                    emit(lambda: nc.vector.tensor_mul(SC[:], W[:], SC[:]))
                    emit(lambda: nc.vector.tensor_scalar_add(SC[:], SC[:], 1.0))
                    n = E_
                    cur, other = SC, SC2
                    while n > 1:
                        h = n // 2
                        ce = cur[:, 0 : 2 * h].rearrange("p (h two) -> p h two", two=2)
                        emit(lambda ce=ce, other=other, h=h: nc.vector.tensor_tensor(
                            out=other[:, 0:h], in0=ce[:, :, 0:1], in1=ce[:, :, 1:2],
                            op=AL.mult))
                        if n % 2:
                            emit(lambda cur=cur, other=other, n=n: nc.vector.tensor_mul(
                                other[:, 0:1], other[:, 0:1], cur[:, n - 1 : n]))
                        cur, other = other, cur
                        n = h
                    emit(lambda cur=cur: nc.vector.tensor_single_scalar(
                        out=SG[:], in_=cur[:, 0:1], scalar=0.0, op=AL.is_gt))
                    emit(lambda cur=cur: nc.vector.tensor_single_scalar(
                        out=PR[:], in_=cur[:, 0:1], scalar=0.0, op=AL.is_lt))
                    emit(lambda: nc.vector.tensor_sub(SG[:], SG[:], PR[:]))
                    emit(lambda: nc.vector.tensor_scalar(
                        out=MI[:], in0=MI[:], scalar1=BC[:], scalar2=0.0,
                        op0=AL.subtract, op1=AL.max))
                    emit(lambda b=b: nc.vector.tensor_mul(OT[:, b : b + 1], SG[:], MI[:]))
                vector.wait_ge(s_v, cnt[0])
                nc.vector.engine_nop().then_inc(s_t, 1)

    return nc


def _kernel_dense(x, bias, inf_mask):
    Bn, E_ = x.shape
    m = np.ascontiguousarray(np.asarray(inf_mask), np.float32)
    Ec = -(-E_ // NCORES)
    PT = 128
    ntiles = -(-Ec // PT)
    Ec = ntiles * PT if Ec > PT else Ec
    key = ("dense", Bn, E_, Ec)
    if key not in _NC_CACHE:
        _NC_CACHE[key] = _build_dense_nc(Bn, E_, Ec)
    nc = _NC_CACHE[key]

    in_maps = []
    for c in range(NCORES):
        lo = c * Ec
        rows = np.full((Ec, E_), np.float32(np.inf), np.float32)
        bcol = np.zeros((Ec, 1), np.float32)
        hi = min(lo + Ec, E_)
        if hi > lo:
            rows[: hi - lo] = m[lo:hi]
            bcol[: hi - lo, 0] = bias[0, lo:hi]
        in_maps.append(
            {
                "mrows": rows,
                "xfull": np.ascontiguousarray(x, np.float32),
                "brows": bcol,
            }
        )

    results = _run_spmd(nc, in_maps)

    out = np.empty((Bn, E_), np.float32)
    for c in range(NCORES):
        lo = c * Ec
        hi = min(lo + Ec, E_)
        if hi > lo:
            out[:, lo:hi] = results[c]["ys"][: hi - lo].T
    return out

